# revision 1
# baseline (speedup 1.0000x reference)
"""Trainium2 Bass kernel for SegmentationAugmentation (3D affine grid_sample, trilinear, border).

Contract: kernel(input_g, label_g, transform) -> (aug_inp f32 [8,1,128,128,128],
                                                  aug_lab bool [8,1,128,128,128])

Math (derived from the reference, with the swapaxes(-3,-1) pairs folded into
index bookkeeping; all spatial dims are 128):

  out[b,c,i,j,k] = trilinear sample of input_g[b,c,:,:,:] at positions
      p-axis (axis 2): U(i,j) = clip(64*(a00*xn(i)+a01*xn(j)+a03)+63.5, 0, 127)
      q-axis (axis 3): V(i,j) = clip(64*(a10*xn(i)+a11*xn(j)+a13)+63.5, 0, 127)
      r-axis (axis 4): W(k)   = clip(64*(a22*xn(k)+a23)+63.5, 0, 127)
  with xn(t) = (2t+1)/128 - 1 and theta = transform[:3].

This relies on the generator's z-rotation structure (theta[0:2,2]==0,
theta[2,0:2]==0), which makes U,V independent of k and W independent of (i,j).
A pure-host fallback handles arbitrary transforms.

Device pipeline, data-parallel over batch (core b handles batch b), fp16
internally (tolerance is 2e-2 rel; fp16 keeps |err| ~1e-2 for the f32 output
and ~4e-3 for labels, patched by the exact host fixup). All DMA layouts are
p-major so every descriptor is a >=4KB per-partition contiguous run:
  1. cast-load (SWDGE): f32 volume -> fp16 vt tiles; partition = p,
     16-q-row chunks, one 8KB-src descriptor per partition.
  2. z-interp (DVE, fp16 2x): Z[p,q,k] = V[..,r0(k)]*(1-fw) + V[..,r1(k)]*fw
     via run-segmented staircase slices into per-volume zt buffers.
  3. Z store (SP HWDGE): zt -> zpad DRAM, one 32KB descriptor per partition.
     Pad rows (p>=128, hit by border-clamped indices with zero weight) are
     zero-filled once per program.
  4. dma_gather (SWDGE): per output column (i,j), two 512B gathers fetch rows
     (p0,q0),(p0,q0+1) and (p0+1,q0),(p0+1,q0+1) of Z. Elements are ordered
     j-major so gather output partitions = i.
  5. combine (DVE, fp16 2x): out = w00*A0 + w01*A1 + w10*B0 + w11*B1 with
     weights stored as duplicated fp16 pairs ([i, j, 2] tables read via a
     [..,[2,GPC],[0,64],[1,2]] AP) so every operand keeps last-dim stride 1
     and the DVE 2x mode stays active.
  6. out store (ACT HWDGE): acc[i, j-chunk, k] -> DRAM, 4KB per-partition
     descriptors.
Outputs are fp16; the host casts aug to f32 and thresholds the label, with
voxels within 6e-3 of 0.5 recomputed in the reference's exact arithmetic.
"""
import numpy as np

N = 128
NVOX = N * N * N
NROWS = N * N           # 16384 (p,q) rows per volume
NPAD = 16640            # padded Z rows (>= 16384 + 129, multiple of 128)
NIDX = 1024             # gather indices per dma_gather call (SWDGE desc ring holds 1024)
GPC = NIDX // 128       # groups (j-values) per gather call = 16
NCALLS = NROWS // NIDX  # 8 gather calls per corner pair per volume
COLS = NIDX // 16       # idx table columns consumed per gather call = 128
ELEM = 256              # gathered element: 2 rows = 256 f16 = 512B
STEP = 128              # row stride in f16 elements
CH = 32                 # q-rows per vt load group
NGRP = NROWS // 128 // CH  # 8 z-groups per volume
NVT = 6                 # vt load buffer depth
NAB = 6                 # gather A/B tile buffer depth
LAGC = 5                # combine headway behind the v1 z stream

_CACHE = {}


def _host_tables(theta):
    """All transform-derived tables, computed in float64 from f32 theta."""
    th = theta.astype(np.float64)
    t = np.arange(N, dtype=np.float64)
    xn = (2.0 * t + 1.0) / N - 1.0

    U = np.clip(64.0 * (th[0, 0] * xn[:, None] + th[0, 1] * xn[None, :] + th[0, 3]) + 63.5, 0.0, 127.0)
    V = np.clip(64.0 * (th[1, 0] * xn[:, None] + th[1, 1] * xn[None, :] + th[1, 3]) + 63.5, 0.0, 127.0)
    W = np.clip(64.0 * (th[2, 2] * xn + th[2, 3]) + 63.5, 0.0, 127.0)

    p0 = np.floor(U).astype(np.int64)
    q0 = np.floor(V).astype(np.int64)
    r0 = np.floor(W).astype(np.int64)
    fu = (U - p0).astype(np.float32)
    fv = (V - q0).astype(np.float32)
    fw = (W - r0).astype(np.float32)
    r1 = np.minimum(r0 + 1, N - 1)

    idxA = (p0 * 128 + q0).astype(np.int16)          # [i,j]
    w00 = ((1 - fu) * (1 - fv)).astype(np.float32)
    w01 = ((1 - fu) * fv).astype(np.float32)
    w10 = (fu * (1 - fv)).astype(np.float32)
    w11 = (fu * fv).astype(np.float32)

    # z-run decomposition: maximal segments where both r0 and r1 step by a
    # constant d in {-1,0,1}
    runs = []
    k = 0
    while k < N:
        step = 0
        if k + 1 < N:
            d = int(r0[k + 1] - r0[k])
            if d == int(r1[k + 1] - r1[k]) and d in (-1, 0, 1):
                step = d
        ln = 1
        while (k + ln < N
               and int(r0[k + ln] - r0[k]) == step * ln
               and int(r1[k + ln] - r1[k]) == step * ln):
            ln += 1
        runs.append((k, ln, int(r0[k]), int(r1[k]), step))
        k += ln

    return dict(idxA=idxA, w00=w00, w01=w01, w10=w10, w11=w11, fw=fw, runs=runs)


def _pack_idxs(idx_flat):
    """int16 dma_gather index layout: element i at [i%16, i//16], replicated to 128 partitions."""
    t = idx_flat.reshape(-1, 16).T.astype(np.int16)  # [16, n/16]
    return np.ascontiguousarray(np.tile(t, (8, 1)))  # [128, n/16]


def _pack_wdup(tables):
    """Duplicated-pair fp16 weight tables [4, i, j*2]: each w[i,j] stored twice
    so the combine AP's last dim is [1,2] (keeps DVE 2x packed mode)."""
    arr = np.stack([tables[c] for c in ("w00", "w01", "w10", "w11")])  # [4, i, j]
    d = np.repeat(arr.astype(np.float16)[..., None], 2, axis=-1)
    return np.ascontiguousarray(d.reshape(4, N, 2 * N))


def _pack_fwrep(tables):
    """fp16 z-lerp weights [2, 128, 128]: (1-fw) and fw replicated across partitions."""
    f0 = (1.0 - tables["fw"].astype(np.float64)).astype(np.float16)
    f1 = tables["fw"].astype(np.float16)
    return np.ascontiguousarray(np.stack([np.tile(f0, (N, 1)), np.tile(f1, (N, 1))]))


def _common_inputs(tables):
    """Per-core constant input tensors (index tables + weights)."""
    # gather elements ordered j-major (m = j*128 + i) so out partitions = i
    idxA_flat = tables["idxA"].T.reshape(-1)
    return {
        "idxA": _pack_idxs(idxA_flat),
        "idxB": _pack_idxs((idxA_flat + 128).astype(np.int16)),
        "wts16": _pack_wdup(tables),
        "fwrep16": _pack_fwrep(tables),
    }


def _build_program(tables, variant="full", reps=1):
    """Raw-Bass (explicit semaphore) program, fp16 internal dtype.

    Engine streams:
      sync   (SP HWDGE):  const loads, zpad pad-row zero fill, Z writes
      gpsimd (SWDGE):     cast-loads (f32->f16) + dma_gather x2 per call
      vector (DVE):       z-interp, combine (fp16 2x packed mode)
      scalar (ACT HWDGE): output writes
    """
    import concourse.bass as bass
    from concourse import bacc, mybir

    runs = tables["runs"]
    f32 = mybir.dt.float32
    f16 = mybir.dt.float16
    i16 = mybir.dt.int16

    nc = bacc.Bacc("TRN2", target_bir_lowering=False, debug=False, num_devices=8)

    vol_in = [nc.dram_tensor(f"vol{v}", [NROWS, N], f32, kind="ExternalInput") for v in range(2)]
    idx_dram = [nc.dram_tensor(nm, [128, NROWS // 16], i16, kind="ExternalInput")
                for nm in ("idxA", "idxB")]
    wts = nc.dram_tensor("wts16", [4, N, 2 * N], f16, kind="ExternalInput")
    fwrep = nc.dram_tensor("fwrep16", [2, N, N], f16, kind="ExternalInput")
    vol_out = [nc.dram_tensor(f"out{v}", [NROWS, N], f16, kind="ExternalOutput") for v in range(2)]
    zpad = [nc.dram_tensor(f"zpad{v}", [NPAD, N], f16, kind="Internal") for v in range(2)]

    AP = bass.AP

    idx_t = [nc.alloc_sbuf_tensor(f"idx{c}_t", [128, NROWS // 16], i16) for c in range(2)]
    w_t = [nc.alloc_sbuf_tensor(f"w{c}_t", [128, 2 * N], f16) for c in range(4)]
    fw_t = [nc.alloc_sbuf_tensor(f"fw{c}_t", [128, N], f16) for c in range(2)]
    vt = [nc.alloc_sbuf_tensor(f"vt{s}", [128, CH * N], f16) for s in range(NVT)]
    ztv = [nc.alloc_sbuf_tensor(f"zt{v}", [128, NROWS], f16) for v in range(2)]
    zero_t = nc.alloc_sbuf_tensor("zero_t", [128, 2 * N], f16)
    At = [nc.alloc_sbuf_tensor(f"At{s}", [128, GPC * ELEM], f16) for s in range(NAB)]
    Bt = [nc.alloc_sbuf_tensor(f"Bt{s}", [128, GPC * ELEM], f16) for s in range(NAB)]
    acc = [nc.alloc_sbuf_tensor(f"acc{s}", [128, GPC * N], f16) for s in range(2)]
    mt = [nc.alloc_sbuf_tensor(f"m{s}", [128, GPC * N], f16) for s in range(4)]
    ztmp = nc.alloc_sbuf_tensor("ztmp", [128, CH * N], f16)

    nrows_ap = (NPAD * N - ELEM) // STEP + 1

    from contextlib import ExitStack
    with ExitStack() as _sctx:
        block = _sctx.enter_context(nc.Block())
        s_idx = _sctx.enter_context(nc.semaphore("s_idx"))
        s_wf = _sctx.enter_context(nc.semaphore("s_wf"))
        s_zero = _sctx.enter_context(nc.semaphore("s_zero"))
        s_pad = _sctx.enter_context(nc.semaphore("s_pad"))
        s_l = [_sctx.enter_context(nc.semaphore(f"s_l{p}")) for p in range(NVT)]
        s_dve_z = _sctx.enter_context(nc.semaphore("s_dve_z"))
        s_zw = [_sctx.enter_context(nc.semaphore(f"s_zw{v}")) for v in range(2)]
        s_gA = [_sctx.enter_context(nc.semaphore(f"s_gA{p}")) for p in range(NAB)]
        s_gB = [_sctx.enter_context(nc.semaphore(f"s_gB{p}")) for p in range(NAB)]
        s_comb = _sctx.enter_context(nc.semaphore("s_comb"))
        s_v = _sctx.enter_context(nc.semaphore("s_v"))
        s_o = [_sctx.enter_context(nc.semaphore(f"s_o{p}")) for p in range(2)]

        do_gather = variant in ("full", "nocomb", "noout")
        do_comb = variant in ("full", "noout")
        do_out = variant == "full"

        @block.sync
        def _(sync):
            for c in range(2):
                sync.dma_start(idx_t[c].ap(), idx_dram[c].ap()).then_inc(s_idx, 16)
            for c in range(4):
                sync.dma_start(w_t[c].ap(), AP(wts, c * N * 2 * N, [[2 * N, 128], [1, 2 * N]])).then_inc(s_wf, 16)
            for c in range(2):
                sync.dma_start(fw_t[c].ap(), AP(fwrep, c * N * N, [[N, 128], [1, N]])).then_inc(s_wf, 16)
            # zero-fill zpad pad rows (p >= 128) once; gathers of clamped
            # border indices read them with zero weight
            sync.wait_ge(s_zero, 1)
            for v in range(2):
                sync.dma_start(
                    AP(zpad[v], NROWS * N, [[N, 128], [128 * N, 2], [1, N]]),
                    AP(zero_t, 0, [[2 * N, 128], [N, 2], [1, N]]),
                ).then_inc(s_pad, 16)
            for r in range(reps):
                for v in range(2):
                    for g in range(NGRP):
                        sync.wait_ge(s_dve_z, NGRP * (r * 2 + v) + g + 1)
                        if g == 0 and r >= 1 and do_comb:
                            # zpad WAR vs previous rep's gather consumers
                            sync.wait_ge(s_comb, (r - 1) * 2 * NCALLS + (v + 1) * NCALLS)
                        elif g == 0 and r >= 1 and variant == "nocomb":
                            glast = ((r - 1) * 2 + v) * NCALLS + NCALLS - 1
                            sync.wait_ge(s_gA[glast % NAB], 16 * (glast // NAB + 1))
                            sync.wait_ge(s_gB[glast % NAB], 16 * (glast // NAB + 1))
                        sync.dma_start(
                            AP(zpad[v], g * CH * N, [[128 * N, 128], [1, CH * N]]),
                            AP(ztv[v], g * CH * N, [[NROWS, 128], [1, CH * N]]),
                        ).then_inc(s_zw[v], 16)
            if variant == "full":
                sync.wait_ge(s_o[0], 16 * NCALLS * reps)
                sync.wait_ge(s_o[1], 16 * NCALLS * reps)
            elif variant == "noout":
                sync.wait_ge(s_comb, 2 * NCALLS * reps)
            elif variant == "nocomb":
                total = 2 * NCALLS * reps
                for p in range(NAB):
                    cnt = 16 * len([g for g in range(total) if g % NAB == p])
                    sync.wait_ge(s_gA[p], cnt)
                    sync.wait_ge(s_gB[p], cnt)
            else:
                sync.wait_ge(s_zw[1], 16 * NGRP * reps)

        @block.gpsimd
        def _(gpsimd):
            nreg = gpsimd.to_reg(NIDX)
            if do_gather:
                gpsimd.wait_ge(s_idx, 32)
                gpsimd.wait_ge(s_pad, 32)

            def castload(r, v, g):
                t = (r * 2 + v) * NGRP + g
                if t >= NVT - 1:
                    gpsimd.wait_ge(s_dve_z, t - (NVT - 1))
                gpsimd.dma_start(
                    AP(vt[t % NVT], 0, [[CH * N, 128], [1, CH * N]]),
                    AP(vol_in[v], g * CH * N, [[128 * N, 128], [1, CH * N]]),
                ).then_inc(s_l[t % NVT], 16)

            def gathers(r, v, call):
                gc = (r * 2 + v) * NCALLS + call
                if gc >= NAB and do_comb:
                    gpsimd.wait_ge(s_comb, gc - (NAB - 1))
                sv = AP(zpad[v], 0, [[STEP, nrows_ap], [1, ELEM]])
                for corner, dst, sg in ((0, At[gc % NAB], s_gA), (1, Bt[gc % NAB], s_gB)):
                    gpsimd.dma_gather(
                        AP(dst, 0, [[GPC * ELEM, 128], [ELEM, GPC], [1, ELEM]]),
                        sv,
                        AP(idx_t[corner], call * COLS, [[NROWS // 16, 128], [1, COLS]]),
                        NIDX, nreg, ELEM, elem_step=STEP,
                    ).then_inc(sg[gc % NAB], 16)

            if not do_gather:
                for r in range(reps):
                    for v in range(2):
                        for g in range(NGRP):
                            castload(r, v, g)
                return
            # Pool ordering: a castload parked behind gathers whose combines
            # transitively need it deadlocks, so v1 loads lead-in before the
            # v0 gather stream and the rest interleave with it; next-rep v0
            # loads interleave with the v1 gather stream.
            PERG = NCALLS // NGRP  # gather calls per z-group
            for g in range(NGRP):
                castload(0, 0, g)
            castload(0, 1, 0)
            castload(0, 1, 1)
            for r in range(reps):
                gpsimd.wait_ge(s_zw[0], 16 * NGRP * (r + 1))
                for call in range(NCALLS):
                    gathers(r, 0, call)
                    if call % PERG == 0 and call // PERG + 2 < NGRP:
                        castload(r, 1, call // PERG + 2)
                gpsimd.wait_ge(s_zw[1], 16 * NGRP * (r + 1))
                for call in range(NCALLS):
                    gathers(r, 1, call)
                    # next rep's v0 loads then its first two v1 loads, all
                    # interleaved before call 6: later gathers pace against
                    # next-rep combines, and a castload queued behind those
                    # would starve the z stream they feed (deadlock)
                    if r + 1 < reps and call < NGRP:
                        castload(r + 1, 0, call)
                    elif r + 1 < reps and call in (NGRP, NGRP + 1):
                        castload(r + 1, 1, call - NGRP)

        @block.vector
        def _(vector):
            mult = mybir.AluOpType.mult
            # DVE self-sync: the DVE pipeline does not interlock same-engine
            # RAW hazards, so each mult-phase -> add-phase transition waits on
            # a self-semaphore (s_v). VC mirrors its value.
            VC = [0]

            def vsync(last_ins):
                last_ins.then_inc(s_v, 1)
                VC[0] += 1
                vector.wait_ge(s_v, VC[0])

            def zgroup(r, v, g):
                t = (r * 2 + v) * NGRP + g
                if t >= 1:
                    # DVE pipeline WAR: prior group's adds must drain before
                    # this group's mults overwrite zt/ztmp
                    vector.wait_ge(s_dve_z, t)
                vector.wait_ge(s_l[t % NVT], 16 * (t // NVT + 1))
                if r >= 1 and g == 0:
                    # ztv[v] WAR vs previous rep's Z write
                    vector.wait_ge(s_zw[v], 16 * NGRP * r)
                s = vt[t % NVT]
                zt = ztv[v]
                last_ins = None
                for (ks, ln, r0s, r1s, st) in runs:
                    zdst = AP(zt, g * CH * N + ks, [[NROWS, 128], [N, CH], [1, ln]])
                    tdst = AP(ztmp, ks, [[CH * N, 128], [N, CH], [1, ln]])
                    v0 = AP(s, r0s, [[CH * N, 128], [N, CH], [st, ln]])
                    v1 = AP(s, r1s, [[CH * N, 128], [N, CH], [st, ln]])
                    f0 = AP(fw_t[0], ks, [[N, 128], [0, CH], [1, ln]])
                    f1 = AP(fw_t[1], ks, [[N, 128], [0, CH], [1, ln]])
                    vector.tensor_tensor(zdst, v0, f0, mult)
                    last_ins = vector.tensor_tensor(tdst, v1, f1, mult)
                vsync(last_ins)
                for (ks, ln, r0s, r1s, st) in runs:
                    zdst = AP(zt, g * CH * N + ks, [[NROWS, 128], [N, CH], [1, ln]])
                    tdst = AP(ztmp, ks, [[CH * N, 128], [N, CH], [1, ln]])
                    last_ins = vector.tensor_add(zdst, zdst, tdst)
                last_ins.then_inc(s_dve_z, 1)

            def combine(r, v, call):
                gc = (r * 2 + v) * NCALLS + call
                if gc >= 1:
                    # DVE pipeline WAR on the mt temps vs previous combine
                    vector.wait_ge(s_comb, gc)
                vector.wait_ge(s_gA[gc % NAB], 16 * (gc // NAB + 1))
                vector.wait_ge(s_gB[gc % NAB], 16 * (gc // NAB + 1))
                if gc >= 2 and do_out:
                    vector.wait_ge(s_o[gc % 2], 16 * (gc // 2))
                A, B, o = At[gc % NAB], Bt[gc % NAB], acc[gc % 2]
                # data APs split the contiguous 128-k run as [2,64],[1,2] so
                # the shape matches the dup-pair weight AP ([0,64],[1,2] =
                # same weight along k, pairs packed -> DVE 2x stays on)
                shp = [[GPC * ELEM, 128], [ELEM, GPC], [2, 64], [1, 2]]
                mshp = [[GPC * N, 128], [N, GPC], [2, 64], [1, 2]]
                oshp = [[GPC * N, 128], [N, GPC], [1, N]]

                def wb(c):
                    return AP(w_t[c], call * GPC * 2, [[2 * N, 128], [2, GPC], [0, 64], [1, 2]])
                maps = [AP(m, 0, mshp) for m in mt]
                vector.tensor_tensor(maps[0], AP(A, 0, shp), wb(0), mult)
                vector.tensor_tensor(maps[1], AP(A, N, shp), wb(1), mult)
                vector.tensor_tensor(maps[2], AP(B, 0, shp), wb(2), mult)
                vsync(vector.tensor_tensor(maps[3], AP(B, N, shp), wb(3), mult))
                flat = [AP(m, 0, oshp) for m in mt]
                vector.tensor_add(flat[0], flat[0], flat[1])
                vsync(vector.tensor_add(flat[2], flat[2], flat[3]))
                vector.tensor_add(AP(o, 0, oshp), flat[0], flat[2]).then_inc(s_comb, 1)

            vector.wait_ge(s_wf, 96)
            vector.memset(zero_t.ap(), 0.0).then_inc(s_zero, 1)
            PERG = NCALLS // NGRP
            LAG = LAGC  # combines of headway behind the z stream (hides the
                        # Z-store + first-gather latency of each volume)
            for r in range(reps):
                for g in range(NGRP):
                    zgroup(r, 0, g)
                    if do_comb and r >= 1:
                        for c in range(PERG * g, PERG * (g + 1)):
                            combine(r - 1, 1, c)
                cn = 0
                for g in range(NGRP):
                    zgroup(r, 1, g)
                    if do_comb:
                        while cn <= PERG * (g + 1) - 1 - LAG:
                            combine(r, 0, cn)
                            cn += 1
                if do_comb:
                    while cn < NCALLS:
                        combine(r, 0, cn)
                        cn += 1
            if do_comb:
                for call in range(NCALLS):
                    combine(reps - 1, 1, call)

        @block.scalar
        def _(scalar):
            if not do_out:
                return
            for r in range(reps):
              for v in range(2):
                for call in range(NCALLS):
                    gc = (r * 2 + v) * NCALLS + call
                    scalar.wait_ge(s_comb, gc + 1)
                    scalar.dma_start(
                        AP(vol_out[v], call * GPC * N, [[128 * N, 128], [N, GPC], [1, N]]),
                        AP(acc[gc % 2], 0, [[GPC * N, 128], [N, GPC], [1, N]]),
                    ).then_inc(s_o[gc % 2], 16)

    nc.compile()
    return nc


def _exact_label_fixup(label_g, theta, lab_f, out_bool, eps=np.float32(6e-3)):
    """Recompute voxels of |lab_f - 0.5| < eps in the reference's exact f32
    arithmetic order (validated bit-exact against the jax reference)."""
    cand = np.abs(lab_f - np.float32(0.5)) < eps
    if not cand.any():
        return out_bool
    bb, ii, jj, kk = np.nonzero(cand.reshape(-1, N, N, N))
    v = _exact_reference_values(label_g, theta, bb, ii, jj, kk)
    out_bool.reshape(-1, N, N, N)[bb, ii, jj, kk] = v > np.float32(0.5)
    return out_bool


def _exact_reference_values(vol_g, theta, bb, ii, jj, kk):
    """Reference-order f32 trilinear values at selected voxels.

    Replicates: grid einsum (x*t0 + y*t1 + z*t2, left-assoc f32) + t3; unnorm;
    8-corner accumulation in (z,y,x) order with w=(wz*wy)*wx, out += v*w.
    """
    f32 = np.float32
    t = np.arange(N, dtype=f32)
    xn = ((f32(2.0) * t + f32(1.0)) / f32(N) - f32(1.0)).astype(f32)
    th = theta.astype(f32)

    x = xn[ii]; y = xn[jj]; z = xn[kk]

    # f32 fma via f64 (exact up to negligible double-rounding corner cases)
    def fma32(a, b, c):
        return (np.float64(a) * np.float64(b) + c.astype(np.float64)).astype(f32)

    # grid components — XLA CPU lowers the einsum as an FMA chain (verified
    # bit-exact): fma(z, t2, fma(y, t1, x*t0)) + t3
    def comp(r):
        a = fma32(y, th[r, 1], (x * th[r, 0]).astype(f32))
        a = fma32(z, th[r, 2], a)
        return (a + th[r, 3]).astype(f32)
    gx, gy, gz = comp(0), comp(1), comp(2)

    def unnorm(c):
        return np.clip(((c + f32(1.0)) * f32(N) - f32(1.0)) * f32(0.5), f32(0.0), f32(N - 1))
    ux, uy, uz = unnorm(gx), unnorm(gy), unnorm(gz)
    x0 = np.floor(ux); y0 = np.floor(uy); z0 = np.floor(uz)
    fx = (ux - x0).astype(f32); fy = (uy - y0).astype(f32); fz = (uz - z0).astype(f32)
    x0i = x0.astype(np.int64); y0i = y0.astype(np.int64); z0i = z0.astype(np.int64)
    x1i = np.minimum(x0i + 1, N - 1); y1i = np.minimum(y0i + 1, N - 1); z1i = np.minimum(z0i + 1, N - 1)

    vol = vol_g.reshape(-1, N, N, N)
    out = np.zeros(bb.shape, f32)
    one = f32(1.0)
    for zi, wz in ((z0i, (one - fz).astype(f32)), (z1i, fz)):
        for yi, wy in ((y0i, (one - fy).astype(f32)), (y1i, fy)):
            for xi, wx in ((x0i, (one - fx).astype(f32)), (x1i, fx)):
                # inp[b, c, zi, yi, xi] in transposed space == vol[b, xi, yi, zi]
                vals = vol[bb, xi, yi, zi]
                w = ((wz * wy).astype(f32) * wx).astype(f32)
                out = (out + (vals * w).astype(f32)).astype(f32)
    return out


def _host_fallback(input_g, label_g, transform):
    """Arbitrary-transform fallback: full reference computation on host."""
    bb, ii, jj, kk = np.meshgrid(np.arange(8), np.arange(N), np.arange(N), np.arange(N), indexing="ij")
    bb, ii, jj, kk = (a.reshape(-1) for a in (bb, ii, jj, kk))
    theta = transform[:3].astype(np.float32)
    aug_inp = _exact_reference_values(input_g, theta, bb, ii, jj, kk).reshape(8, 1, N, N, N)
    lab = _exact_reference_values(label_g, theta, bb, ii, jj, kk).reshape(8, 1, N, N, N)
    return aug_inp.astype(np.float32), lab > np.float32(0.5)


def kernel(input_g, label_g, transform):
    input_g = np.ascontiguousarray(input_g, dtype=np.float32)
    label_g = np.ascontiguousarray(label_g, dtype=np.float32)
    transform = np.asarray(transform, dtype=np.float32)
    theta = transform[:3]

    structured = (abs(float(theta[0, 2])) < 1e-12 and abs(float(theta[1, 2])) < 1e-12
                  and abs(float(theta[2, 0])) < 1e-12 and abs(float(theta[2, 1])) < 1e-12)
    if not structured:
        return _host_fallback(input_g, label_g, transform)

    from concourse.bass_utils import run_bass_kernel_spmd

    tables = _host_tables(theta)
    key = transform.tobytes()
    if key not in _CACHE:
        _CACHE[key] = _build_program(tables)
    nc = _CACHE[key]

    common = _common_inputs(tables)
    in_maps = []
    for b in range(8):
        in_maps.append(dict(common,
                            vol0=input_g[b, 0].reshape(NROWS, N),
                            vol1=label_g[b, 0].reshape(NROWS, N)))

    res = run_bass_kernel_spmd(nc, in_maps, core_ids=list(range(8)))

    aug_inp = np.empty((8, 1, N, N, N), np.float32)
    lab_f = np.empty((8, 1, N, N, N), np.float32)
    for b in range(8):
        aug_inp[b, 0] = res.results[b]["out0"].astype(np.float32).reshape(N, N, N)
        lab_f[b, 0] = res.results[b]["out1"].astype(np.float32).reshape(N, N, N)

    out_bool = lab_f > np.float32(0.5)
    out_bool = _exact_label_fixup(label_g, theta, lab_f, out_bool)
    return aug_inp, out_bool



# revision 12
# speedup vs baseline: 3.1822x; 3.1822x over previous
"""Trainium2 Bass kernel for SegmentationAugmentation (3D affine grid_sample, trilinear, border).

Contract: kernel(input_g, label_g, transform) -> (aug_inp f32 [8,1,128,128,128],
                                                  aug_lab bool [8,1,128,128,128])

Math (derived from the reference, with the swapaxes(-3,-1) pairs folded into
index bookkeeping; all spatial dims are 128):

  out[b,c,i,j,k] = trilinear sample of input_g[b,c,:,:,:] at positions
      p-axis (axis 2): U(i,j) = clip(64*(a00*xn(i)+a01*xn(j)+a03)+63.5, 0, 127)
      q-axis (axis 3): V(i,j) = clip(64*(a10*xn(i)+a11*xn(j)+a13)+63.5, 0, 127)
      r-axis (axis 4): W(k)   = clip(64*(a22*xn(k)+a23)+63.5, 0, 127)
  with xn(t) = (2t+1)/128 - 1 and theta = transform[:3].

This relies on the generator's z-rotation structure (theta[0:2,2]==0,
theta[2,0:2]==0), which makes U,V independent of k and W independent of (i,j).
A pure-host fallback handles arbitrary transforms.

v2 design (vs v1): the dma_gather stage is per-ELEMENT-cost bound (~2-4ns per
gathered element regardless of 0.5-2KB size), so both volumes' z-interped data
are INTERLEAVED row-wise in one DRAM tensor zpadB, with row-unit
u = (p*128+q)*2 + v. One 1KB gather element (idx = anchor row a, elem_step =
2 row-units) then covers rows (q0,v0),(q0,v1),(q0+1,v0),(q0+1,v1) at one p --
both volumes' A-taps in a single element. This HALVES gather element count.

Device pipeline, data-parallel over batch (core b handles batch b), fp16
internally:
  1. cast-load (SWDGE): f32 volume -> fp16 vt tiles; partition = p,
     32-q-row groups per volume (8 loads/rep).
  2. z-interp (DVE, fp16 2x): ztv[p, q*256 + v*128 + k] =
     V[..,r0(k)]*(1-fw) + V[..,r1(k)]*fw via run-segmented staircase slices.
     The v-interleaved SBUF layout makes the Z store fully contiguous.
  3. Z store (SP HWDGE): ztv -> zpadB[r%2] DRAM, 16KB/partition descriptors.
     Pad row-units (>= 2*NROWS, hit by border-clamped idx with zero weight)
     are zero-filled once.
  4. dma_gather (SWDGE): per output column (i,j), TWO 1KB gathers fetch
     (p0,q0..q0+1,v0..v1) and (p0+1,...). j-major element order so output
     partitions = i.
  5. combine (DVE, fp16 2x): per (call, v): out = w00*A0+w01*A1+w10*B0+w11*B1
     with dup-pair fp16 weight tables (shared across v).
  6. out store (ACT HWDGE): acc -> DRAM, 2KB/partition descriptors.
Outputs are fp16; the host casts aug to f32 and thresholds the label, with
voxels within 6e-3 of 0.5 recomputed in the reference's exact arithmetic.
"""
import numpy as np

N = 128
NVOX = N * N * N
NROWS = N * N           # 16384 (p,q) rows per volume
NPAD = 16640            # padded anchor rows (>= 16384 + 129, multiple of 128)
NIDX = 1024             # gather indices per dma_gather call
GPC = NIDX // 128       # j-values per gather call = 8
NCALLS = NROWS // NIDX  # 16 gather calls per corner stream per rep
ELEM = 512              # gathered element: 2 anchors x 2 vols = 512 f16 = 1KB
ESTEP = 256             # element index stride in f16 (one anchor = 2 row-units)
CH = 32                 # q-rows per vt load group
NGRP = NROWS // 128 // CH  # 4 z-groups per volume
NVT = 3                 # vt load buffer depth
NAB = 5                 # gather A/B tile buffer depth

_CACHE = {}


def _host_tables(theta):
    """All transform-derived tables, computed in float64 from f32 theta."""
    th = theta.astype(np.float64)
    t = np.arange(N, dtype=np.float64)
    xn = (2.0 * t + 1.0) / N - 1.0

    U = np.clip(64.0 * (th[0, 0] * xn[:, None] + th[0, 1] * xn[None, :] + th[0, 3]) + 63.5, 0.0, 127.0)
    V = np.clip(64.0 * (th[1, 0] * xn[:, None] + th[1, 1] * xn[None, :] + th[1, 3]) + 63.5, 0.0, 127.0)
    W = np.clip(64.0 * (th[2, 2] * xn + th[2, 3]) + 63.5, 0.0, 127.0)

    p0 = np.floor(U).astype(np.int64)
    q0 = np.floor(V).astype(np.int64)
    r0 = np.floor(W).astype(np.int64)
    fu = (U - p0).astype(np.float32)
    fv = (V - q0).astype(np.float32)
    fw = (W - r0).astype(np.float32)
    r1 = np.minimum(r0 + 1, N - 1)

    idxA = (p0 * 128 + q0).astype(np.int16)          # [i,j]
    w00 = ((1 - fu) * (1 - fv)).astype(np.float32)
    w01 = ((1 - fu) * fv).astype(np.float32)
    w10 = (fu * (1 - fv)).astype(np.float32)
    w11 = (fu * fv).astype(np.float32)

    # z-run decomposition: maximal segments where both r0 and r1 step by a
    # constant d in {-1,0,1}
    runs = []
    k = 0
    while k < N:
        step = 0
        if k + 1 < N:
            d = int(r0[k + 1] - r0[k])
            if d == int(r1[k + 1] - r1[k]) and d in (-1, 0, 1):
                step = d
        ln = 1
        while (k + ln < N
               and int(r0[k + ln] - r0[k]) == step * ln
               and int(r1[k + ln] - r1[k]) == step * ln):
            ln += 1
        runs.append((k, ln, int(r0[k]), int(r1[k]), step))
        k += ln

    return dict(idxA=idxA, w00=w00, w01=w01, w10=w10, w11=w11, fw=fw, runs=runs)


def _pack_idxs(idx_flat):
    """int16 dma_gather index layout: element i at [i%16, i//16], replicated to 128 partitions."""
    t = idx_flat.reshape(-1, 16).T.astype(np.int16)  # [16, n/16]
    return np.ascontiguousarray(np.tile(t, (8, 1)))  # [128, n/16]


def _pack_wdup(tables):
    """Duplicated-pair fp16 weight tables [4, i, j*2]: each w[i,j] stored twice
    so the combine AP's last dim is [1,2] (keeps DVE 2x packed mode)."""
    arr = np.stack([tables[c] for c in ("w00", "w01", "w10", "w11")])  # [4, i, j]
    d = np.repeat(arr.astype(np.float16)[..., None], 2, axis=-1)
    return np.ascontiguousarray(d.reshape(4, N, 2 * N))


def _pack_fwrep(tables):
    """fp16 z-lerp weights [2, 128, 128]: (1-fw) and fw replicated across partitions."""
    f0 = (1.0 - tables["fw"].astype(np.float64)).astype(np.float16)
    f1 = tables["fw"].astype(np.float16)
    return np.ascontiguousarray(np.stack([np.tile(f0, (N, 1)), np.tile(f1, (N, 1))]))


def _common_inputs(tables):
    """Per-core constant input tensors (index tables + weights)."""
    # gather elements ordered j-major (m = j*128 + i) so out partitions = i
    idxA_flat = tables["idxA"].T.reshape(-1)
    return {
        "idxA": _pack_idxs(idxA_flat),
        "idxB": _pack_idxs((idxA_flat + 128).astype(np.int16)),
        "wts16": _pack_wdup(tables),
        "fwrep16": _pack_fwrep(tables),
    }


def _build_program(tables, variant="full", reps=1):
    """Raw-Bass (explicit semaphore) program, fp16 internal dtype.

    Engine streams:
      sync   (SP HWDGE):  const loads, zpadB pad zero fill, Z writes
      gpsimd (SWDGE):     cast-loads (f32->f16) + volume-fused dma_gathers
      vector (DVE):       z-interp, combine (fp16 2x packed mode)
      scalar (ACT HWDGE): output writes
    """
    import concourse.bass as bass
    from concourse import bacc, mybir

    runs = tables["runs"]
    f32 = mybir.dt.float32
    f16 = mybir.dt.float16
    i16 = mybir.dt.int16

    nc = bacc.Bacc("TRN2", target_bir_lowering=False, debug=False, num_devices=8)

    vol_in = [nc.dram_tensor(f"vol{v}", [NROWS, N], f32, kind="ExternalInput") for v in range(2)]
    idx_dram = [nc.dram_tensor(nm, [128, NROWS // 16], i16, kind="ExternalInput")
                for nm in ("idxA", "idxB")]
    wts = nc.dram_tensor("wts16", [4, N, 2 * N], f16, kind="ExternalInput")
    fwrep = nc.dram_tensor("fwrep16", [2, N, N], f16, kind="ExternalInput")
    vol_out = [nc.dram_tensor(f"out{v}", [NROWS, N], f16, kind="ExternalOutput") for v in range(2)]
    # v-interleaved z-volume, double-buffered across reps
    zpadB = [nc.dram_tensor(f"zpadB{b}", [2 * NPAD, N], f16, kind="Internal") for b in range(2)]

    AP = bass.AP

    idx_t = [nc.alloc_sbuf_tensor(f"idx{c}_t", [128, NROWS // 16], i16) for c in range(2)]
    w_t = [nc.alloc_sbuf_tensor(f"w{c}_t", [128, 2 * N], f16) for c in range(4)]
    fw_t = [nc.alloc_sbuf_tensor(f"fw{c}_t", [128, N], f16) for c in range(2)]
    vt = [nc.alloc_sbuf_tensor(f"vt{s}", [128, CH * N], f16) for s in range(NVT)]
    ztv = nc.alloc_sbuf_tensor("ztv", [128, 2 * NROWS], f16)  # [p, q*256 + v*128 + k]
    zero_t = nc.alloc_sbuf_tensor("zero_t", [128, 4 * N], f16)
    At = [nc.alloc_sbuf_tensor(f"At{s}", [128, GPC * ELEM], f16) for s in range(NAB)]
    Bt = [nc.alloc_sbuf_tensor(f"Bt{s}", [128, GPC * ELEM], f16) for s in range(NAB)]
    acc = [[nc.alloc_sbuf_tensor(f"acc{v}_{s}", [128, GPC * N], f16) for s in range(2)]
           for v in range(2)]
    mt = [nc.alloc_sbuf_tensor(f"m{s}", [128, GPC * N], f16) for s in range(4)]
    ztmp = nc.alloc_sbuf_tensor("ztmp", [128, CH * N], f16)

    nrows_ap = (2 * NPAD * N - ELEM) // ESTEP + 1

    do_gather = variant in ("full", "nocomb", "noout")
    do_comb = variant in ("full", "noout")
    do_out = variant == "full"

    from contextlib import ExitStack
    with ExitStack() as _sctx:
        block = _sctx.enter_context(nc.Block())
        s_idx = _sctx.enter_context(nc.semaphore("s_idx"))
        s_wf = _sctx.enter_context(nc.semaphore("s_wf"))
        s_zero = _sctx.enter_context(nc.semaphore("s_zero"))
        s_pad = _sctx.enter_context(nc.semaphore("s_pad"))
        s_l = [_sctx.enter_context(nc.semaphore(f"s_l{p}")) for p in range(NVT)]
        s_dve_z = _sctx.enter_context(nc.semaphore("s_dve_z"))
        s_zw = _sctx.enter_context(nc.semaphore("s_zw"))
        s_gA = [_sctx.enter_context(nc.semaphore(f"s_gA{p}")) for p in range(NAB)]
        s_gB = [_sctx.enter_context(nc.semaphore(f"s_gB{p}")) for p in range(NAB)]
        s_comb = _sctx.enter_context(nc.semaphore("s_comb"))
        s_v = _sctx.enter_context(nc.semaphore("s_v"))
        s_o = [[_sctx.enter_context(nc.semaphore(f"s_o{v}_{p}")) for p in range(2)]
               for v in range(2)]

        # ---- per-rep counts -------------------------------------------------
        ZG = 2 * NGRP          # 8 zgroups per rep (2 vols x 4 groups)
        CB = 2 * NCALLS        # 32 combines per rep (16 calls x 2 vols)

        @block.sync
        def _(sync):
            for c in range(2):
                sync.dma_start(idx_t[c].ap(), idx_dram[c].ap()).then_inc(s_idx, 16)
            for c in range(4):
                sync.dma_start(w_t[c].ap(), AP(wts, c * N * 2 * N, [[2 * N, 128], [1, 2 * N]])).then_inc(s_wf, 16)
            for c in range(2):
                sync.dma_start(fw_t[c].ap(), AP(fwrep, c * N * N, [[N, 128], [1, N]])).then_inc(s_wf, 16)
            # zero-fill pad row-units (u >= 2*NROWS) once; gathers of clamped
            # border indices read them with zero weight
            sync.wait_ge(s_zero, 1)
            for b in range(2):
                sync.dma_start(
                    AP(zpadB[b], 2 * NROWS * N, [[4 * N, 128], [1, 4 * N]]),
                    zero_t.ap(),
                ).then_inc(s_pad, 16)
            for r in range(reps):
                for g in range(NGRP):
                    # both volumes' z-interp for group g done
                    sync.wait_ge(s_dve_z, ZG * r + 2 * (g + 1))
                    if g == 0 and r >= 2 and (do_comb or variant == "nocomb"):
                        # zpadB[r%2] WAR vs rep r-2's gather consumers
                        if do_comb:
                            sync.wait_ge(s_comb, CB * (r - 1))
                        else:
                            glast = (r - 2) * NCALLS + NCALLS - 1
                            sync.wait_ge(s_gA[glast % NAB], 16 * (glast // NAB + 1))
                            sync.wait_ge(s_gB[glast % NAB], 16 * (glast // NAB + 1))
                    sync.dma_start(
                        AP(zpadB[r % 2], g * CH * 2 * N, [[2 * NROWS, 128], [1, CH * 2 * N]]),
                        AP(ztv, g * CH * 2 * N, [[2 * NROWS, 128], [1, CH * 2 * N]]),
                    ).then_inc(s_zw, 16)
            if variant == "full":
                for v in range(2):
                    for p in range(2):
                        sync.wait_ge(s_o[v][p], 16 * (NCALLS // 2) * reps)
            elif variant == "noout":
                sync.wait_ge(s_comb, CB * reps)
            elif variant == "nocomb":
                total = NCALLS * reps
                for p in range(NAB):
                    cnt = 16 * len([g for g in range(total) if g % NAB == p])
                    sync.wait_ge(s_gA[p], cnt)
                    sync.wait_ge(s_gB[p], cnt)
            else:
                sync.wait_ge(s_zw, 16 * NGRP * reps)

        @block.gpsimd
        def _(gpsimd):
            nreg = gpsimd.to_reg(NIDX)
            if do_gather:
                gpsimd.wait_ge(s_idx, 32)
                gpsimd.wait_ge(s_pad, 32)

            def castload(r, g, v):
                t = r * ZG + 2 * g + v   # matches zgroup consumption order
                if t >= NVT:
                    # vt ring WAR: the zgroup that previously used this slot
                    gpsimd.wait_ge(s_dve_z, t - (NVT - 1))
                gpsimd.dma_start(
                    AP(vt[t % NVT], 0, [[CH * N, 128], [1, CH * N]]),
                    AP(vol_in[v], g * CH * N, [[128 * N, 128], [1, CH * N]]),
                ).then_inc(s_l[t % NVT], 16)

            def gathers(r, call):
                gc = r * NCALLS + call
                if gc >= NAB and do_comb:
                    # At/Bt ring WAR vs the combine pair that read this slot
                    gpsimd.wait_ge(s_comb, 2 * (gc - (NAB - 1)))
                sv = AP(zpadB[r % 2], 0, [[ESTEP, nrows_ap], [1, ELEM]])
                for corner, dst, sg in ((0, At[gc % NAB], s_gA), (1, Bt[gc % NAB], s_gB)):
                    gpsimd.dma_gather(
                        AP(dst, 0, [[GPC * ELEM, 128], [ELEM, GPC], [1, ELEM]]),
                        sv,
                        AP(idx_t[corner], call * (NIDX // 16), [[NROWS // 16, 128], [1, NIDX // 16]]),
                        NIDX, nreg, ELEM, elem_step=ESTEP,
                    ).then_inc(sg[gc % NAB], 16)

            # lead-in: rep 0's castloads
            for g in range(NGRP):
                for v in range(2):
                    castload(0, g, v)
            if not do_gather:
                for r in range(1, reps):
                    for g in range(NGRP):
                        for v in range(2):
                            castload(r, g, v)
                return
            for r in range(reps):
                # all 4 Z stores of rep r complete before its gathers
                gpsimd.wait_ge(s_zw, 16 * NGRP * (r + 1))
                for call in range(NCALLS):
                    gathers(r, call)
                    # pack next rep's 8 castloads into the first 4 calls so
                    # they are all queued before any gather that waits on
                    # this rep's combines (call >= NAB-1); a castload parked
                    # behind such a gather would deadlock the zg(r+1) ->
                    # comb(r) DVE order
                    if r + 1 < reps and call < NGRP:
                        castload(r + 1, call, 0)
                        castload(r + 1, call, 1)

        @block.vector
        def _(vector):
            mult = mybir.AluOpType.mult
            # DVE self-sync: the DVE pipeline does not interlock same-engine
            # RAW hazards, so each mult-phase -> add-phase transition waits on
            # a self-semaphore (s_v). VC mirrors its value.
            VC = [0]

            def vsync(last_ins):
                last_ins.then_inc(s_v, 1)
                VC[0] += 1
                vector.wait_ge(s_v, VC[0])

            def zgroup(r, g, v):
                t = r * ZG + 2 * g + v     # issue order: (g, v) inner v
                if t >= 1:
                    # DVE pipeline WAR: prior group's adds must drain before
                    # this group's mults overwrite ztmp
                    vector.wait_ge(s_dve_z, t)
                vector.wait_ge(s_l[t % NVT], 16 * (t // NVT + 1))
                if r >= 1 and t % ZG == 0:
                    # ztv WAR vs previous rep's Z stores
                    vector.wait_ge(s_zw, 16 * NGRP * r)
                s = vt[t % NVT]
                base = g * CH * 2 * N + v * N
                last_ins = None
                for (ks, ln, r0s, r1s, st) in runs:
                    zdst = AP(ztv, base + ks, [[2 * NROWS, 128], [2 * N, CH], [1, ln]])
                    tdst = AP(ztmp, ks, [[CH * N, 128], [N, CH], [1, ln]])
                    v0 = AP(s, r0s, [[CH * N, 128], [N, CH], [st, ln]])
                    v1 = AP(s, r1s, [[CH * N, 128], [N, CH], [st, ln]])
                    f0 = AP(fw_t[0], ks, [[N, 128], [0, CH], [1, ln]])
                    f1 = AP(fw_t[1], ks, [[N, 128], [0, CH], [1, ln]])
                    vector.tensor_tensor(zdst, v0, f0, mult)
                    last_ins = vector.tensor_tensor(tdst, v1, f1, mult)
                vsync(last_ins)
                for (ks, ln, r0s, r1s, st) in runs:
                    zdst = AP(ztv, base + ks, [[2 * NROWS, 128], [2 * N, CH], [1, ln]])
                    tdst = AP(ztmp, ks, [[CH * N, 128], [N, CH], [1, ln]])
                    last_ins = vector.tensor_add(zdst, zdst, tdst)
                last_ins.then_inc(s_dve_z, 1)

            def combine(r, call, v):
                gc = r * NCALLS + call
                cc = 2 * gc + v            # global combine index
                if cc >= 1:
                    # DVE pipeline WAR on the mt temps vs previous combine
                    vector.wait_ge(s_comb, cc)
                vector.wait_ge(s_gA[gc % NAB], 16 * (gc // NAB + 1))
                vector.wait_ge(s_gB[gc % NAB], 16 * (gc // NAB + 1))
                oc = r * NCALLS + call     # per-vol store index
                if oc >= 2 and do_out:
                    # acc[v][call%2] WAR vs the store that last read it
                    vector.wait_ge(s_o[v][call % 2], 16 * (oc // 2))
                A, B, o = At[gc % NAB], Bt[gc % NAB], acc[v][call % 2]
                # data APs split each 128-k block as [2,64],[1,2] so the shape
                # matches the dup-pair weight AP (same weight along k, pairs
                # packed -> DVE 2x stays on); v picks the 128-f16 sub-block
                shp0 = [[GPC * ELEM, 128], [ELEM, GPC], [2, 64], [1, 2]]
                mshp = [[GPC * N, 128], [N, GPC], [2, 64], [1, 2]]
                oshp = [[GPC * N, 128], [N, GPC], [1, N]]

                def wb(c):
                    return AP(w_t[c], call * GPC * 2, [[2 * N, 128], [2, GPC], [0, 64], [1, 2]])
                maps = [AP(m, 0, mshp) for m in mt]
                vector.tensor_tensor(maps[0], AP(A, v * N, shp0), wb(0), mult)
                vector.tensor_tensor(maps[1], AP(A, 2 * N + v * N, shp0), wb(1), mult)
                vector.tensor_tensor(maps[2], AP(B, v * N, shp0), wb(2), mult)
                vsync(vector.tensor_tensor(maps[3], AP(B, 2 * N + v * N, shp0), wb(3), mult))
                flat = [AP(m, 0, oshp) for m in mt]
                vector.tensor_add(flat[0], flat[0], flat[1])
                vsync(vector.tensor_add(flat[2], flat[2], flat[3]))
                vector.tensor_add(AP(o, 0, oshp), flat[0], flat[2]).then_inc(s_comb, 1)

            vector.wait_ge(s_wf, 96)
            vector.memset(zero_t.ap(), 0.0).then_inc(s_zero, 1)
            # order: zg(0); [zg(r+1); comb(r)] for r in 0..reps-1.  zg(r+1)
            # runs while gathers(r) stream (its castloads are packed into the
            # first 4 gather calls); comb(r) then chases the gather stream.
            for g in range(NGRP):
                for v in range(2):
                    zgroup(0, g, v)
            for r in range(reps):
                if r + 1 < reps:
                    for g in range(NGRP):
                        for v in range(2):
                            zgroup(r + 1, g, v)
                if do_comb:
                    for call in range(NCALLS):
                        for v in range(2):
                            combine(r, call, v)

        @block.scalar
        def _(scalar):
            if not do_out:
                return
            for r in range(reps):
                for call in range(NCALLS):
                    for v in range(2):
                        cc = 2 * (r * NCALLS + call) + v
                        scalar.wait_ge(s_comb, cc + 1)
                        scalar.dma_start(
                            AP(vol_out[v], call * GPC * N, [[128 * N, 128], [N, GPC], [1, N]]),
                            AP(acc[v][call % 2], 0, [[GPC * N, 128], [N, GPC], [1, N]]),
                        ).then_inc(s_o[v][call % 2], 16)

    nc.compile()
    return nc


def _exact_label_fixup(label_g, theta, lab_f, out_bool, eps=np.float32(6e-3)):
    """Recompute voxels of |lab_f - 0.5| < eps in the reference's exact f32
    arithmetic order (validated bit-exact against the jax reference)."""
    cand = np.abs(lab_f - np.float32(0.5)) < eps
    if not cand.any():
        return out_bool
    bb, ii, jj, kk = np.nonzero(cand.reshape(-1, N, N, N))
    v = _exact_reference_values(label_g, theta, bb, ii, jj, kk)
    out_bool.reshape(-1, N, N, N)[bb, ii, jj, kk] = v > np.float32(0.5)
    return out_bool


def _exact_reference_values(vol_g, theta, bb, ii, jj, kk):
    """Reference-order f32 trilinear values at selected voxels.

    Replicates: grid einsum (x*t0 + y*t1 + z*t2, left-assoc f32) + t3; unnorm;
    8-corner accumulation in (z,y,x) order with w=(wz*wy)*wx, out += v*w.
    """
    f32 = np.float32
    t = np.arange(N, dtype=f32)
    xn = ((f32(2.0) * t + f32(1.0)) / f32(N) - f32(1.0)).astype(f32)
    th = theta.astype(f32)

    x = xn[ii]; y = xn[jj]; z = xn[kk]

    # f32 fma via f64 (exact up to negligible double-rounding corner cases)
    def fma32(a, b, c):
        return (np.float64(a) * np.float64(b) + c.astype(np.float64)).astype(f32)

    # grid components — XLA CPU lowers the einsum as an FMA chain (verified
    # bit-exact): fma(z, t2, fma(y, t1, x*t0)) + t3
    def comp(r):
        a = fma32(y, th[r, 1], (x * th[r, 0]).astype(f32))
        a = fma32(z, th[r, 2], a)
        return (a + th[r, 3]).astype(f32)
    gx, gy, gz = comp(0), comp(1), comp(2)

    def unnorm(c):
        return np.clip(((c + f32(1.0)) * f32(N) - f32(1.0)) * f32(0.5), f32(0.0), f32(N - 1))
    ux, uy, uz = unnorm(gx), unnorm(gy), unnorm(gz)
    x0 = np.floor(ux); y0 = np.floor(uy); z0 = np.floor(uz)
    fx = (ux - x0).astype(f32); fy = (uy - y0).astype(f32); fz = (uz - z0).astype(f32)
    x0i = x0.astype(np.int64); y0i = y0.astype(np.int64); z0i = z0.astype(np.int64)
    x1i = np.minimum(x0i + 1, N - 1); y1i = np.minimum(y0i + 1, N - 1); z1i = np.minimum(z0i + 1, N - 1)

    vol = vol_g.reshape(-1, N, N, N)
    out = np.zeros(bb.shape, f32)
    one = f32(1.0)
    for zi, wz in ((z0i, (one - fz).astype(f32)), (z1i, fz)):
        for yi, wy in ((y0i, (one - fy).astype(f32)), (y1i, fy)):
            for xi, wx in ((x0i, (one - fx).astype(f32)), (x1i, fx)):
                # inp[b, c, zi, yi, xi] in transposed space == vol[b, xi, yi, zi]
                vals = vol[bb, xi, yi, zi]
                w = ((wz * wy).astype(f32) * wx).astype(f32)
                out = (out + (vals * w).astype(f32)).astype(f32)
    return out


def _host_fallback(input_g, label_g, transform):
    """Arbitrary-transform fallback: full reference computation on host."""
    bb, ii, jj, kk = np.meshgrid(np.arange(8), np.arange(N), np.arange(N), np.arange(N), indexing="ij")
    bb, ii, jj, kk = (a.reshape(-1) for a in (bb, ii, jj, kk))
    theta = transform[:3].astype(np.float32)
    aug_inp = _exact_reference_values(input_g, theta, bb, ii, jj, kk).reshape(8, 1, N, N, N)
    lab = _exact_reference_values(label_g, theta, bb, ii, jj, kk).reshape(8, 1, N, N, N)
    return aug_inp.astype(np.float32), lab > np.float32(0.5)


def kernel(input_g, label_g, transform):
    input_g = np.ascontiguousarray(input_g, dtype=np.float32)
    label_g = np.ascontiguousarray(label_g, dtype=np.float32)
    transform = np.asarray(transform, dtype=np.float32)
    theta = transform[:3]

    structured = (abs(float(theta[0, 2])) < 1e-12 and abs(float(theta[1, 2])) < 1e-12
                  and abs(float(theta[2, 0])) < 1e-12 and abs(float(theta[2, 1])) < 1e-12)
    if not structured:
        return _host_fallback(input_g, label_g, transform)

    from concourse.bass_utils import run_bass_kernel_spmd

    tables = _host_tables(theta)
    key = transform.tobytes()
    if key not in _CACHE:
        _CACHE[key] = _build_program(tables)
    nc = _CACHE[key]

    common = _common_inputs(tables)
    in_maps = []
    for b in range(8):
        in_maps.append(dict(common,
                            vol0=input_g[b, 0].reshape(NROWS, N),
                            vol1=label_g[b, 0].reshape(NROWS, N)))

    res = run_bass_kernel_spmd(nc, in_maps, core_ids=list(range(8)))

    aug_inp = np.empty((8, 1, N, N, N), np.float32)
    lab_f = np.empty((8, 1, N, N, N), np.float32)
    for b in range(8):
        aug_inp[b, 0] = res.results[b]["out0"].astype(np.float32).reshape(N, N, N)
        lab_f[b, 0] = res.results[b]["out1"].astype(np.float32).reshape(N, N, N)

    out_bool = lab_f > np.float32(0.5)
    out_bool = _exact_label_fixup(label_g, theta, lab_f, out_bool)
    return aug_inp, out_bool
